# revision 4
# baseline (speedup 1.0000x reference)
"""Multi-head self-attention with RoPE on 8 Trainium2 NeuronCores.

Sharding: core c = batch*2 + head_group. Each core computes 8 of the 16
heads for one batch element end-to-end (QKV projection, RoPE, attention,
out-projection partial). Host sums the two head-group partials per batch
and applies the (linear) bias corrections.

Matmuls run in bf16 with fp32 PSUM accumulation, EXCEPT the scores
matmuls, which use fp8e4m3 DoubleRow at half the cycle cost: the
contraction (d=64) is packed as 64 partitions x 2 planes, where the k
side is exact (plane0 = fp8 hi, plane1 = fp8 residual lo, summed by the
DoubleRow accumulation) and the q side is a single fp8 quantization
duplicated across both planes with a stride-0 access pattern.  Only q's
~2.7%-rms quantization error survives, which the softmax damps to
~1.2e-2 on the final output (measured 1.6e-2 total against a 2e-2 gate).
DoubleRow on this walrus build requires base partition 0, so RoPE'd q/k
are reshuffled from the [2 heads x 64d, T] projection layout into
[64d, head, T] tiles on partitions 0-63 with SBUF->SBUF DMAs.

Softmax skips the max-subtraction (scores are bounded) and folds the
row-sum into the P@V matmul via a ones-column on V; the reciprocal of
the row-sum runs on the DVE (keeping the Act engine exp-only), and the
row broadcast is an off-engine DRAM round-trip.

Scheduling: a single interleaved emission stream keeps the PE busy
continuously. Projection/V/out-projection granules are pumped between
scores tiles as filler (credit-based pacing), and the P@V accumulation
lags the exp by several tiles so the PE never waits on the Act engine.
"""

import numpy as np
import ml_dtypes

# ---------------------------------------------------------------------------
# Workaround: this walrus build rejects >1 sem-wait on a CTRL-only (Drain)
# instruction. TileContext's tail drain carries one wait per outstanding
# logical proc; split them across a chain of single-wait drains.
# ---------------------------------------------------------------------------
_PATCHED = False


def _split_waits_json(raw: bytes) -> bytes:
    """Split instructions carrying >1 sem-wait into single-wait NoOp
    carriers followed by the original instruction (this walrus build
    allows at most one sync-wait per instruction)."""
    import json

    m = json.loads(raw)

    def fix_block(bb):
        insts = bb.get("instructions")
        if not isinstance(insts, list):
            return
        out = []
        for inst in insts:
            si = inst.get("sync_info") if isinstance(inst, dict) else None
            waits = si.get("on_wait") if si else None
            if waits and len(waits) > 1:
                for k, w in enumerate(waits[:-1]):
                    out.append({
                        "debug": inst.get("debug"),
                        "engine": inst["engine"],
                        "ins": [], "outs": [],
                        "name": f'{inst["name"]}_wc{k}',
                        "opcode": "NoOp",
                        "sync_info": {"on_update": [], "on_wait": [w]},
                        "text_hint": "waitsplit",
                    })
                si["on_wait"] = [waits[-1]]
            out.append(inst)
        bb["instructions"] = out

    def walk(obj):
        if isinstance(obj, dict):
            if "instructions" in obj:
                fix_block(obj)
            for v in obj.values():
                walk(v)
        elif isinstance(obj, list):
            for v in obj:
                walk(v)

    walk(m)
    return json.dumps(m).encode()


def _apply_tile_patch():
    global _PATCHED
    if _PATCHED:
        return
    import concourse.bass as bass

    orig = bass.Bass.to_json_bytes

    def to_json_bytes_split(self, *a, **kw):
        return _split_waits_json(orig(self, *a, **kw))

    bass.Bass.to_json_bytes = to_json_bytes_split
    _PATCHED = True


# ---------------------------------------------------------------------------
# Problem dims (hardcoded for the full problem; parameterized for testing)
# ---------------------------------------------------------------------------
class Cfg:
    def __init__(self, T=2048, CIN=1024, JH=512, CO=1024, D=64):
        self.T, self.CIN, self.JH, self.CO, self.D = T, CIN, JH, CO, D
        self.H = JH // D            # heads per core
        self.NCC = CIN // 128       # contraction chunks
        self.NJ = JH // 128         # q/k row tiles
        self.NT = T // 128          # t partition tiles (= s chunks)
        self.TC = 512               # matmul moving-dim chunk
        self.NTC = T // self.TC
        assert JH % 128 == 0 and CIN % 128 == 0 and T % self.TC == 0
        assert D == 64, "RoPE layout assumes D=64 (pairs at +-32 partitions)"


def rope_tables(cfg, dtype=np.float32):
    """cos/sin tables laid out for the [j-within-tile, t] orientation.

    Partition p of a q/k row-tile holds head-channel d = p % 64; the RoPE
    pair of d is d^32 within the same 64-block. sin is sign-baked:
    negative for the first half of each head, positive for the second.
    """
    half = cfg.D // 2
    theta = (10000.0 ** (-np.arange(half, dtype=np.float32) / half)).astype(np.float32)
    t = np.arange(cfg.T, dtype=np.float32)
    freqs = t[None, :] * theta[:, None]          # (32, T) fp32, matches reference
    cos32, sin32 = np.cos(freqs), np.sin(freqs)
    cos = np.tile(cos32, (4, 1))                 # (128, T)
    sgn = np.where((np.arange(128) % 64) < 32, -1.0, 1.0).astype(np.float32)
    sin = np.tile(sin32, (4, 1)) * sgn[:, None]
    return cos.astype(dtype), sin.astype(dtype)


def perm_matrix():
    """[128,128] permutation: out[p] = in[sigma(p)], sigma(p) = p^32 in 64-blocks."""
    m = np.zeros((128, 128), dtype=np.float32)
    k = np.arange(128)
    sigma = (k // 64) * 64 + (k + 32) % 64
    m[k, sigma] = 1.0
    return m.astype(ml_dtypes.bfloat16)


# ---------------------------------------------------------------------------
# Bass program
# ---------------------------------------------------------------------------
def build_nc(cfg, with_qk_bias=False):
    _apply_tile_patch()
    import concourse.bass as bass
    import concourse.tile as tile
    from concourse import mybir
    import contextlib

    f32 = mybir.dt.float32
    bf16 = mybir.dt.bfloat16
    f8 = mybir.dt.float8e4
    DR = mybir.MatmulPerfMode.DoubleRow
    nc = bass.Bass()

    xT = nc.dram_tensor("xT", (cfg.CIN, cfg.T), bf16, kind="ExternalInput")
    wqT = nc.dram_tensor("wqT", (cfg.CIN, cfg.JH), bf16, kind="ExternalInput")
    wkT = nc.dram_tensor("wkT", (cfg.CIN, cfg.JH), bf16, kind="ExternalInput")
    wvT = nc.dram_tensor("wvT", (cfg.CIN, cfg.JH), bf16, kind="ExternalInput")
    woT = nc.dram_tensor("woT", (cfg.JH, cfg.CO), bf16, kind="ExternalInput")
    cosT = nc.dram_tensor("cosT", (128, cfg.T), bf16, kind="ExternalInput")
    sinT = nc.dram_tensor("sinT", (128, cfg.T), bf16, kind="ExternalInput")
    permM = nc.dram_tensor("permM", (128, 128), bf16, kind="ExternalInput")
    if with_qk_bias:
        bqD = nc.dram_tensor("bq", (cfg.NJ, 128), f32, kind="ExternalInput")
        bkD = nc.dram_tensor("bk", (cfg.NJ, 128), f32, kind="ExternalInput")
    y = nc.dram_tensor("y", (cfg.T, cfg.CO), f32, kind="ExternalOutput")

    NCC, NJ, NT, TC, NTC, H, D = cfg.NCC, cfg.NJ, cfg.NT, cfg.TC, cfg.NTC, cfg.H, cfg.D
    NSC = NT                      # number of 128-key chunks
    LAG = 7                       # AV lags exp by this many score tiles

    with tile.TileContext(nc) as tc:
        with contextlib.ExitStack() as ctx:
            consts = ctx.enter_context(tc.tile_pool(name="consts", bufs=1))
            slabs = ctx.enter_context(tc.tile_pool(name="slabs", bufs=1))
            evac = ctx.enter_context(tc.tile_pool(name="evac", bufs=3))
            ropetmp = ctx.enter_context(tc.tile_pool(name="ropetmp", bufs=2))
            stage = ctx.enter_context(tc.tile_pool(name="stage", bufs=3))
            ppool = ctx.enter_context(tc.tile_pool(name="ppool", bufs=9))
            ypool = ctx.enter_context(tc.tile_pool(name="ypool", bufs=2))
            rpool = ctx.enter_context(tc.tile_pool(name="rpool", bufs=2))
            avsp = ctx.enter_context(tc.tile_pool(name="avsp", bufs=4))
            rdram = ctx.enter_context(
                tc.tile_pool(name="rdram", bufs=2, space="DRAM"))
            # PSUM: 4 banks scores/outproj ring + 2 banks AV + 2 scratch
            psc = ctx.enter_context(tc.tile_pool(name="psc", bufs=2, space="PSUM"))
            pav = ctx.enter_context(tc.tile_pool(name="pav", bufs=2, space="PSUM"))
            pscr = ctx.enter_context(tc.tile_pool(name="pscr", bufs=2, space="PSUM"))

            # ---- constants ----
            cos_sb = consts.tile([128, cfg.T], bf16)
            sin_sb = consts.tile([128, cfg.T], bf16)
            perm_sb = consts.tile([128, 128], bf16)
            nc.sync.dma_start(out=perm_sb, in_=permM[:, :])
            if with_qk_bias:
                bq_sb = consts.tile([128, NJ], f32)
                bk_sb = consts.tile([128, NJ], f32)
                nc.sync.dma_start(out=bq_sb, in_=bqD[:, :].rearrange("j p -> p j"))
                nc.sync.dma_start(out=bk_sb, in_=bkD[:, :].rearrange("j p -> p j"))

            # ---- weight / activation slabs (DMA order = first-use order) ----
            w_sbs = {}
            for name in ("q", "k", "v"):
                w_sbs[name] = slabs.tile([128, NCC, cfg.JH], bf16, tag=f"w{name}",
                                         name=f"w{name}_sb")
            x_sb = slabs.tile([128, NCC, cfg.T], bf16)
            xT_v = xT[:, :].rearrange("(cc p) t -> p cc t", p=128)
            nc.sync.dma_start(out=w_sbs["q"],
                              in_=wqT[:, :].rearrange("(cc p) j -> p cc j", p=128))
            nc.sync.dma_start(out=x_sb[:, :, 0:TC], in_=xT_v[:, :, 0:TC])
            nc.sync.dma_start(out=w_sbs["k"],
                              in_=wkT[:, :].rearrange("(cc p) j -> p cc j", p=128))
            nc.sync.dma_start(out=cos_sb, in_=cosT[:, :])
            nc.sync.dma_start(out=sin_sb, in_=sinT[:, :])
            nc.sync.dma_start(out=w_sbs["v"],
                              in_=wvT[:, :].rearrange("(cc p) j -> p cc j", p=128))
            for tq in range(1, NTC):
                tql = slice(tq * TC, (tq + 1) * TC)
                nc.sync.dma_start(out=x_sb[:, :, tql], in_=xT_v[:, :, tql])
            wo_sb = slabs.tile([128, NJ, cfg.CO], bf16)
            nc.sync.dma_start(out=wo_sb, in_=woT[:, :].rearrange("(jc p) o -> p jc o", p=128))

            # fp8 scores operands on partitions 0-63 (DoubleRow needs base 0):
            #   q8r[d, h, t]   single-quantized q
            #   khl[d, hi/lo, h, t]  exact k as hi+lo residual pair
            q8r = slabs.tile([64, H, cfg.T], f8, tag="q8r")
            khl = slabs.tile([64, 2, H, cfg.T], f8, tag="khl")
            v_sb = slabs.tile([128, NT, H, D + 1], bf16, tag="vaug")
            ao_sb = slabs.tile([128, NJ, cfg.T], bf16, tag="ao")
            # ones column for the rowsum trick
            nc.gpsimd.memset(v_sb[:, :, :, D:D + 1], 1.0)

            def dup2(ap):
                """Insert a stride-0 plane dim after the partition dim."""
                return bass.AP(
                    tensor=ap.tensor, offset=ap.offset,
                    ap=[list(ap.ap[0]), [0, 2]] + [list(dd) for dd in ap.ap[1:]])

            # ============== granules + deferred-tail scheduler ==============
            # Each granule emits its PE matmuls, then returns a "tail"
            # closure (ops that depend on another engine's evac of those
            # matmuls). Pumped granules run the tail after the NEXT unit's
            # matmuls, so the PE never sits waiting on its own granule's
            # cross-engine round-trip.
            emitted = [0]          # PE cycles emitted so far
            pending = []           # granules with un-run tails


            class G:
                __slots__ = ("fn", "rows", "done", "tail")

                def __init__(self, fn, rows):
                    self.fn, self.rows = fn, rows
                    self.done, self.tail = False, None

                def emit(self, defer=False):
                    if self.done:
                        self.flush()
                        return
                    self.done = True
                    tail = self.fn()
                    if tail is not None:
                        if defer:
                            self.tail = tail
                            pending.append(self)
                        else:
                            tail()

                def flush(self):
                    if self.tail is not None:
                        t, self.tail = self.tail, None
                        pending.remove(self)
                        t()

            def flush_pending(except_g=None):
                for g in pending[:]:
                    if g is not except_g:
                        g.flush()

            def proj_granule(name, jt, tcc):
                """k/q projection + RoPE + fp8 staging for row-tile jt,
                t-chunk tcc.  Output goes to q8r (single fp8) or khl
                (hi/lo fp8 pair) via SBUF->SBUF partition-reshuffle DMAs."""
                tsl = slice(tcc * TC, (tcc + 1) * TC)
                w = w_sbs[name]
                ps = pscr.tile([128, TC], f32, tag="scr", name=f"ps_{name}{jt}_{tcc}")
                for cc in range(NCC):
                    nc.tensor.matmul(
                        ps, lhsT=w[:, cc, jt * 128:(jt + 1) * 128],
                        rhs=x_sb[:, cc, tsl],
                        start=(cc == 0), stop=(cc == NCC - 1))
                if with_qk_bias:
                    b = bq_sb if name == "q" else bk_sb
                    nc.vector.tensor_scalar_add(ps, ps, b[:, jt:jt + 1])
                qb = evac.tile([128, TC], bf16, tag="qb")
                nc.vector.tensor_copy(qb, ps)
                emitted[0] += NCC * TC

                def tail():
                    pp = pscr.tile([128, TC], f32, tag="scr",
                                   name=f"pp_{name}{jt}_{tcc}")
                    nc.tensor.matmul(pp, lhsT=perm_sb, rhs=qb)
                    t1 = ropetmp.tile([128, TC], bf16, tag="t1")
                    nc.vector.tensor_mul(t1, qb, cos_sb[:, tsl])
                    t2 = ropetmp.tile([128, TC], bf16, tag="t2")
                    nc.vector.tensor_mul(t2, pp, sin_sb[:, tsl])
                    if name == "q":
                        qs = stage.tile([128, TC], f8, tag="qs", name="qs")
                        nc.vector.tensor_add(qs, t1, t2)
                        for half in range(2):
                            nc.sync.dma_start(
                                out=q8r[0:64, 2 * jt + half, tsl],
                                in_=qs[64 * half:64 * half + 64, :])
                    else:
                        kb = stage.tile([128, TC], bf16, tag="kb", name="kb")
                        nc.vector.tensor_add(kb, t1, t2)
                        khi = stage.tile([128, TC], f8, tag="khi", name="khi")
                        nc.vector.tensor_copy(khi, kb)
                        klo = stage.tile([128, TC], f8, tag="klo", name="klo")
                        nc.vector.tensor_sub(klo, kb, khi)
                        for half in range(2):
                            nc.sync.dma_start(
                                out=khl[0:64, 0, 2 * jt + half, tsl],
                                in_=khi[64 * half:64 * half + 64, :])
                            nc.sync.dma_start(
                                out=khl[0:64, 1, 2 * jt + half, tsl],
                                in_=klo[64 * half:64 * half + 64, :])
                    emitted[0] += TC

                return tail

            def v_granule(tt):
                """v projection (natural orientation) for t-tile tt."""
                ps = pscr.tile([128, cfg.JH], f32, tag="scr", name=f"pv_{tt}")
                for cc in range(NCC):
                    nc.tensor.matmul(
                        ps, lhsT=x_sb[:, cc, tt * 128:(tt + 1) * 128],
                        rhs=w_sbs["v"][:, cc, :],
                        start=(cc == 0), stop=(cc == NCC - 1))
                nc.vector.tensor_copy(
                    v_sb[:, tt, :, 0:D],
                    ps[:, :].rearrange("p (h d) -> p h d", h=H))
                emitted[0] += NCC * cfg.JH

            def outproj_granule(tt):
                """out-projection for t-tile tt (all CO columns)."""
                tq = tt * 128 * NTC // cfg.T
                for p_ in range(NJ):
                    GNB[(tq, p_)].emit()
                ps = psc.tile([128, cfg.CO], f32, tag="psc", name=f"yps_{tt}")
                for u in range(cfg.CO // TC):
                    for jc in range(NJ):
                        nc.tensor.matmul(
                            ps[:, u * TC:(u + 1) * TC],
                            lhsT=ao_sb[:, jc, tt * 128:(tt + 1) * 128],
                            rhs=wo_sb[:, jc, u * TC:(u + 1) * TC],
                            start=(jc == 0), stop=(jc == NJ - 1))
                yb = ypool.tile([128, cfg.CO], f32, tag="yb", name=f"yb_{tt}")
                nc.vector.tensor_copy(yb, ps)
                nc.sync.dma_start(out=y[tt * 128:(tt + 1) * 128, :], in_=yb)
                emitted[0] += (cfg.CO // TC) * NJ * TC

            GK = {(jt, c): G(lambda n=jt, c_=c: proj_granule("k", n, c_),
                             NCC * TC + TC)
                  for jt in range(NJ) for c in range(NTC)}
            GQ = {(jt, c): G(lambda n=jt, c_=c: proj_granule("q", n, c_),
                             NCC * TC + TC)
                  for jt in range(NJ) for c in range(NTC)}
            GV = {tt: G(lambda t_=tt: v_granule(t_), NCC * cfg.JH)
                  for tt in range(NT)}
            GOP = {tt: G(lambda t_=tt: outproj_granule(t_),
                         (cfg.CO // TC) * NJ * TC) for tt in range(NT)}

            # softmax normalize, split in two phases per slot:
            #   A (DVE): r = 1/rowsum (DVE reciprocal; Act stays exp-only)
            #   B (DMA+DVE): broadcast r over 64 partitions via a DRAM
            #      round-trip, ao = av * rb        — forced a bit later
            norm_state = {}

            def norm_a(avs, pair, tcq):
                st = []
                for half in range(2):
                    # same-partition in/out (lane 64): DVE lanes don't shift
                    rf = rpool.tile([D + 1, TC], f32, tag="rf", name="rf")
                    nc.vector.reciprocal(rf[D:D + 1, :], avs[half][D:D + 1, :])
                    st.append(rf)
                norm_state[(tcq, pair)] = st

            def norm_b(avs, pair, tcq):
                # broadcast r across 64 partitions via a DRAM round-trip
                # (off-engine; fully slack -- ao is consumed a slot later)
                tsl = slice(tcq * TC, (tcq + 1) * TC)
                st = norm_state.pop((tcq, pair))
                for half in range(2):
                    p0 = 64 * half
                    rd = rdram.tile([1, TC], f32, tag="rd", name="rd")
                    nc.sync.dma_start(out=rd, in_=st[half][D:D + 1, :])
                    rd_ap = rd[0:1, :]
                    r_bc = bass.AP(
                        tensor=rd_ap.tensor, offset=rd_ap.offset,
                        ap=[[0, D]] + [list(dd) for dd in rd_ap.ap[1:]])
                    rb = rpool.tile([D, TC], f32, tag="rb", name="rb")
                    nc.sync.dma_start(out=rb, in_=r_bc)
                    nc.vector.tensor_mul(
                        ao_sb[p0:p0 + 64, pair, tsl], avs[half][0:D, :], rb)

            GNB = {}

            filler = []            # list of G, pumped in order
            fill_pos = [0]

            def pump(target):
                while emitted[0] < target and fill_pos[0] < len(filler):
                    g = filler[fill_pos[0]]
                    if g.done:
                        fill_pos[0] += 1
                        continue
                    g.emit(defer=True)
                    flush_pending(except_g=g)
                    fill_pos[0] += 1

            # Static filler order: K row-tiles (chunk-major), V tiles, then
            # Q chunks for later tcq; OP granules appended when available.
            for c in range(NTC):
                for jt in range(NJ):
                    filler.append(GK[(jt, c)])
                    if c == 0 and jt > 0:
                        filler.append(GQ[(jt, 0)])
                for k in range(NT // NTC):
                    filler.append(GV[c * (NT // NTC) + k])
            for c in range(1, NTC):
                for jt in range(NJ):
                    filler.append(GQ[(jt, c)])

            total_rows = (2 * NJ * NTC * (NCC * TC + TC) + NT * NCC * cfg.JH
                          + NSC * NTC * NJ * (TC + 2 * TC)
                          + NT * (cfg.CO // TC) * NJ * TC)
            per_iter = total_rows // (NTC * NJ * NSC) + 1

            # ---------------- main interleaved slot loop ----------------
            slots = [(tcq, p) for tcq in range(NTC) for p in range(NJ)]

            # prefix: first k/q granules so scores can start immediately
            GK[(0, 0)].emit()
            GQ[(0, 0)].emit()

            prev_slot = None        # (avs, pair, tcq) of previous slot
            for si, (tcq, pair) in enumerate(slots):
                jt = pair
                tsl = slice(tcq * TC, (tcq + 1) * TC)
                if prev_slot is not None:
                    norm_a(*prev_slot)
                    pa, pp_, pt_ = prev_slot
                    GNB[(pt_, pp_)] = G(
                        lambda a=pa, b=pp_, c=pt_: norm_b(a, b, c), 0)
                if si > 0 and si % NJ == 0:
                    prev = tcq - 1
                    for k in range(NT // NTC):
                        filler.append(GOP[prev * (NT // NTC) + k])

                avs = [pav.tile([D + 1, TC], f32, tag="av",
                                name=f"av_{tcq}_{pair}_{i}") for i in range(2)]
                p_tiles = {}
                for sc in range(NSC):
                    GK[(jt, sc * 128 // TC)].emit()
                    GQ[(jt, tcq)].emit()
                    pairP = psc.tile([128, 2 * TC], f32, tag="psc",
                                     name=f"sc_{tcq}_{pair}_{sc}")
                    for half in range(2):
                        h = 2 * pair + half
                        nc.tensor.matmul(
                            pairP[:, half * TC:(half + 1) * TC],
                            lhsT=khl[0:64, :, h, sc * 128:(sc + 1) * 128],
                            rhs=dup2(q8r[0:64, h, tsl]),
                            perf_mode=DR)
                    emitted[0] += TC
                    p_sb = ppool.tile([128, 2 * TC], bf16, tag="p",
                                      name=f"p_{tcq}_{pair}_{sc}")
                    nc.scalar.activation(
                        p_sb, pairP, mybir.ActivationFunctionType.Exp,
                        scale=float(1.0 / np.sqrt(D)))
                    p_tiles[sc] = p_sb
                    flush_pending()
                    if sc == 6 and prev_slot is not None:
                        # free the previous slot's AV psum before our AV starts
                        GNB[(prev_slot[2], prev_slot[1])].emit()
                    if sc >= LAG:
                        sa = sc - LAG
                        GV[sa].emit()
                        pb = p_tiles.pop(sa)
                        for half in range(2):
                            nc.tensor.matmul(
                                avs[half],
                                lhsT=v_sb[:, sa, 2 * pair + half, :],
                                rhs=pb[:, half * TC:(half + 1) * TC],
                                start=(sa == 0), stop=(sa == NSC - 1))
                        emitted[0] += 2 * TC
                        if sa + 1 < NSC:
                            GV[sa + 1].emit()
                    # prefetch upcoming k chunks / next slot deps (deferred)
                    if tcq == 0:
                        nxt = min(sc // 4 + 1, NTC - 1)
                        GK[(jt, nxt)].emit(defer=True)
                    if sc == 13 and si + 1 < len(slots):
                        ntcq, npair = slots[si + 1]
                        if ntcq == 0:
                            GK[(npair, 0)].emit(defer=True)
                        GQ[(npair, ntcq)].emit(defer=True)
                    pump((si * NSC + sc + 1) * per_iter)
                for sa in range(NSC - LAG, NSC):
                    GV[sa].emit()
                    pb = p_tiles.pop(sa)
                    for half in range(2):
                        nc.tensor.matmul(
                            avs[half],
                            lhsT=v_sb[:, sa, 2 * pair + half, :],
                            rhs=pb[:, half * TC:(half + 1) * TC],
                            start=(sa == 0), stop=(sa == NSC - 1))
                    emitted[0] += 2 * TC
                avs_sb = []
                for half in range(2):
                    asb = avsp.tile([D + 1, TC], bf16, tag="avs",
                                    name=f"avsb_{tcq}_{pair}_{half}")
                    nc.vector.tensor_copy(asb, avs[half])
                    avs_sb.append(asb)
                prev_slot = (avs_sb, pair, tcq)

            # tail: last slot's normalize + remaining out-proj
            flush_pending()
            norm_a(*prev_slot)
            GNB[(prev_slot[2], prev_slot[1])] = G(
                lambda a=prev_slot[0], b=prev_slot[1], c=prev_slot[2]:
                norm_b(a, b, c), 0)
            GNB[(prev_slot[2], prev_slot[1])].emit()
            for g in filler:
                g.emit()
            flush_pending()
            for tt in range(NT):
                GOP[tt].emit()

    return nc


_NC_CACHE = {}


def _get_nc(cfg, with_qk_bias):
    key = (cfg.T, cfg.CIN, cfg.JH, cfg.CO, cfg.D, with_qk_bias)
    if key not in _NC_CACHE:
        _NC_CACHE[key] = build_nc(cfg, with_qk_bias)
    return _NC_CACHE[key]


def make_in_maps(cfg, x, Wq, bq, Wk, bk, Wv, bv, Wo, bo, n_groups=2):
    """Build the per-core input dicts. Core c = b * n_groups + g."""
    bf = ml_dtypes.bfloat16
    B = x.shape[0]
    cos, sin = rope_tables(cfg, dtype=bf)
    pm = perm_matrix()
    with_qk_bias = bool(np.any(bq) or np.any(bk))
    in_maps = []
    for b in range(B):
        for g in range(n_groups):
            rows = slice(g * cfg.JH, (g + 1) * cfg.JH)
            m = {
                "xT": np.ascontiguousarray(x[b].T).astype(bf),
                "wqT": np.ascontiguousarray(Wq[rows, :].T).astype(bf),
                "wkT": np.ascontiguousarray(Wk[rows, :].T).astype(bf),
                "wvT": np.ascontiguousarray(Wv[rows, :].T).astype(bf),
                "woT": np.ascontiguousarray(Wo[:, rows].T).astype(bf),
                "cosT": cos, "sinT": sin, "permM": pm,
            }
            if with_qk_bias:
                m["bq"] = np.ascontiguousarray(
                    bq[rows].reshape(cfg.NJ, 128).astype(np.float32))
                m["bk"] = np.ascontiguousarray(
                    bk[rows].reshape(cfg.NJ, 128).astype(np.float32))
            in_maps.append(m)
    return in_maps, with_qk_bias


def run(x, Wq, bq, Wk, bk, Wv, bv, Wo, bo, trace=False):
    from concourse.bass_utils import run_bass_kernel_spmd

    B, T, C = x.shape
    n_groups = 2
    cfg = Cfg(T=T, CIN=C, JH=C // n_groups, CO=C, D=64)
    in_maps, with_qk_bias = make_in_maps(
        cfg, x, Wq, bq, Wk, bk, Wv, bv, Wo, bo, n_groups)
    nc = _get_nc(cfg, with_qk_bias)
    res = run_bass_kernel_spmd(
        nc, in_maps, core_ids=list(range(len(in_maps))), trace=trace)
    out = np.zeros((B, T, C), dtype=np.float32)
    for c, r in enumerate(res.results):
        out[c // n_groups] += r["y"]
    # linear bias corrections (exact): v-bias passes through softmax row-sum=1;
    # out-proj bias is additive.
    out += (bv.astype(np.float32) @ Wo.T.astype(np.float32) + bo.astype(np.float32))
    return out, res


def kernel(x, Wq, bq, Wk, bk, Wv, bv, Wo, bo):
    out, _ = run(
        np.asarray(x, dtype=np.float32),
        np.asarray(Wq, dtype=np.float32), np.asarray(bq, dtype=np.float32),
        np.asarray(Wk, dtype=np.float32), np.asarray(bk, dtype=np.float32),
        np.asarray(Wv, dtype=np.float32), np.asarray(bv, dtype=np.float32),
        np.asarray(Wo, dtype=np.float32), np.asarray(bo, dtype=np.float32))
    return out


# revision 17
# speedup vs baseline: 1.2573x; 1.2573x over previous
"""Multi-head self-attention with RoPE on 8 Trainium2 NeuronCores.

Sharding: core c = batch*2 + head_group. Each core computes 8 of the 16
heads for one batch element end-to-end (QKV projection, RoPE, attention,
out-projection partial). Host sums the two head-group partials per batch
and applies the (linear) bias corrections.

All matmuls run in bf16 with fp32 PSUM accumulation. Softmax skips the
max-subtraction (scores for this problem are bounded by a few units, so
exp() is safe in fp32) and folds the row-sum into the P@V matmul via a
ones-column on V; normalization happens on the tiny [64, t] output.

RoPE's cross-channel pairing uses a PERM64-permuted head-channel order
(pairs 16 partitions apart inside a 32-partition quadrant) so the
partner fetch is a single DVE stream_shuffle, with the permutation
folded into the host-side Wq/Wk row order and cos/sin tables.

Scheduling: a single interleaved emission stream keeps the PE busy
continuously. Projection/V/out-projection granules are pumped between
scores tiles as filler (credit-based pacing), the P@V accumulation lags
the exp by several tiles so the PE never waits on the Act engine, and
the softmax normalization is fully decoupled: the AV accumulator is
evacuated to SBUF (freeing its PSUM bank immediately), the reciprocal
runs on all 128 DVE lanes after a DRAM-hop relayout of the rowsum
(keeping the Act engine exp-only), and the row broadcast is an
off-engine DRAM round-trip.
"""

import numpy as np
import ml_dtypes

# ---------------------------------------------------------------------------
# Workaround: this walrus build rejects >1 sem-wait on a CTRL-only (Drain)
# instruction. TileContext's tail drain carries one wait per outstanding
# logical proc; split them across a chain of single-wait drains.
# ---------------------------------------------------------------------------
_PATCHED = False


def _split_waits_json(raw: bytes) -> bytes:
    """Split instructions carrying >1 sem-wait into single-wait NoOp
    carriers followed by the original instruction (this walrus build
    allows at most one sync-wait per instruction)."""
    import json

    m = json.loads(raw)

    def fix_block(bb):
        insts = bb.get("instructions")
        if not isinstance(insts, list):
            return
        out = []
        for inst in insts:
            si = inst.get("sync_info") if isinstance(inst, dict) else None
            waits = si.get("on_wait") if si else None
            if waits and len(waits) > 1:
                for k, w in enumerate(waits[:-1]):
                    out.append({
                        "debug": inst.get("debug"),
                        "engine": inst["engine"],
                        "ins": [], "outs": [],
                        "name": f'{inst["name"]}_wc{k}',
                        "opcode": "NoOp",
                        "sync_info": {"on_update": [], "on_wait": [w]},
                        "text_hint": "waitsplit",
                    })
                si["on_wait"] = [waits[-1]]
            out.append(inst)
        bb["instructions"] = out

    def walk(obj):
        if isinstance(obj, dict):
            if "instructions" in obj:
                fix_block(obj)
            for v in obj.values():
                walk(v)
        elif isinstance(obj, list):
            for v in obj:
                walk(v)

    walk(m)
    return json.dumps(m).encode()


def _apply_tile_patch():
    global _PATCHED
    if _PATCHED:
        return
    import concourse.bass as bass

    orig = bass.Bass.to_json_bytes

    def to_json_bytes_split(self, *a, **kw):
        return _split_waits_json(orig(self, *a, **kw))

    bass.Bass.to_json_bytes = to_json_bytes_split
    _PATCHED = True


# Note: gpsimd InstISA ops (partition_broadcast) and custom DVE ops
# (reciprocal_approx_fast) fail walrus codegen on this build ("ISA wrong
# length" in visitInstISA) -- only standard BIR ops are used below.


# ---------------------------------------------------------------------------
# Problem dims (hardcoded for the full problem; parameterized for testing)
# ---------------------------------------------------------------------------
class Cfg:
    def __init__(self, T=2048, CIN=1024, JH=512, CO=1024, D=64):
        self.T, self.CIN, self.JH, self.CO, self.D = T, CIN, JH, CO, D
        self.H = JH // D            # heads per core
        self.NCC = CIN // 128       # contraction chunks
        self.NJ = JH // 128         # q/k row tiles
        self.NT = T // 128          # t partition tiles (= s chunks)
        self.TC = 512               # matmul moving-dim chunk
        self.NTC = T // self.TC
        assert JH % 128 == 0 and CIN % 128 == 0 and T % self.TC == 0
        assert D == 64, "RoPE layout assumes D=64 (pairs at +-32 partitions)"


# Head-channel permutation: within each head's 64 channels, order them
# [0..15, 32..47, 16..31, 48..63].  Each RoPE pair (d, d+32) then sits 16
# partitions apart INSIDE one 32-partition quadrant, so the partner fetch
# is a single DVE stream_shuffle (mask i -> (i+16)%32) instead of a PE
# permutation matmul.  Scores contract q.k over d, so any per-head channel
# permutation applied consistently to Wq and Wk rows is exact.
PERM64 = np.concatenate([
    np.arange(0, 16), np.arange(32, 48),
    np.arange(16, 32), np.arange(48, 64)])


def rope_tables(cfg, dtype=np.float32):
    """cos/sin tables for the PERM64-permuted [j-within-tile, t] layout.

    Partition p of a q/k row-tile holds head-channel d = PERM64[p % 64];
    theta index is d % 32.  sin is sign-baked: negative for x1-type
    channels (d < 32, i.e. p%32 < 16), positive for x2-type.
    """
    half = cfg.D // 2
    theta = (10000.0 ** (-np.arange(half, dtype=np.float32) / half)).astype(np.float32)
    t = np.arange(cfg.T, dtype=np.float32)
    freqs = t[None, :] * theta[:, None]          # (32, T) fp32, matches reference
    cos32, sin32 = np.cos(freqs), np.sin(freqs)
    d128 = PERM64[np.arange(128) % 64]
    cos = cos32[d128 % 32, :]                    # (128, T)
    sgn = np.where(d128 < 32, -1.0, 1.0).astype(np.float32)
    sin = sin32[d128 % 32, :] * sgn[:, None]
    return cos.astype(dtype), sin.astype(dtype)


# ---------------------------------------------------------------------------
# Bass program
# ---------------------------------------------------------------------------
def build_nc(cfg, with_qk_bias=False):
    _apply_tile_patch()
    import concourse.bass as bass
    import concourse.tile as tile
    from concourse import mybir
    import contextlib

    f32 = mybir.dt.float32
    bf16 = mybir.dt.bfloat16
    nc = bass.Bass()

    xT = nc.dram_tensor("xT", (cfg.CIN, cfg.T), bf16, kind="ExternalInput")
    wqT = nc.dram_tensor("wqT", (cfg.CIN, cfg.JH), bf16, kind="ExternalInput")
    wkT = nc.dram_tensor("wkT", (cfg.CIN, cfg.JH), bf16, kind="ExternalInput")
    wvT = nc.dram_tensor("wvT", (cfg.CIN, cfg.JH), bf16, kind="ExternalInput")
    woT = nc.dram_tensor("woT", (cfg.JH, cfg.CO), bf16, kind="ExternalInput")
    cosT = nc.dram_tensor("cosT", (128, cfg.T), bf16, kind="ExternalInput")
    sinT = nc.dram_tensor("sinT", (128, cfg.T), bf16, kind="ExternalInput")
    if with_qk_bias:
        bqD = nc.dram_tensor("bq", (cfg.NJ, 128), f32, kind="ExternalInput")
        bkD = nc.dram_tensor("bk", (cfg.NJ, 128), f32, kind="ExternalInput")
    y = nc.dram_tensor("y", (cfg.T, cfg.CO), f32, kind="ExternalOutput")

    NCC, NJ, NT, TC, NTC, H, D = cfg.NCC, cfg.NJ, cfg.NT, cfg.TC, cfg.NTC, cfg.H, cfg.D
    NSC = NT                      # number of 128-key chunks
    LAG = 7                       # AV lags exp by this many score tiles
    SHUF_MASK = [(i + 16) % 32 for i in range(32)]

    with tile.TileContext(nc) as tc:
        with contextlib.ExitStack() as ctx:
            consts = ctx.enter_context(tc.tile_pool(name="consts", bufs=1))
            slabs = ctx.enter_context(tc.tile_pool(name="slabs", bufs=1))
            evac = ctx.enter_context(tc.tile_pool(name="evac", bufs=3))
            ropetmp = ctx.enter_context(tc.tile_pool(name="ropetmp", bufs=2))
            ppool = ctx.enter_context(tc.tile_pool(name="ppool", bufs=9))
            ypool = ctx.enter_context(tc.tile_pool(name="ypool", bufs=2))
            rpool = ctx.enter_context(tc.tile_pool(name="rpool", bufs=2))
            avsp = ctx.enter_context(tc.tile_pool(name="avsp", bufs=4))
            # rd2 tiles live from norm_a until the next slot's norm_b read,
            # so 2 halves x 2 slots must coexist
            rdram = ctx.enter_context(
                tc.tile_pool(name="rdram", bufs=4, space="DRAM"))
            # PSUM: 4 banks scores/outproj ring + 2 banks AV + 2 scratch
            psc = ctx.enter_context(tc.tile_pool(name="psc", bufs=2, space="PSUM"))
            pav = ctx.enter_context(tc.tile_pool(name="pav", bufs=2, space="PSUM"))
            pscr = ctx.enter_context(tc.tile_pool(name="pscr", bufs=2, space="PSUM"))

            # ---- constants ----
            cos_sb = consts.tile([128, cfg.T], bf16)
            sin_sb = consts.tile([128, cfg.T], bf16)
            if with_qk_bias:
                bq_sb = consts.tile([128, NJ], f32)
                bk_sb = consts.tile([128, NJ], f32)
                nc.sync.dma_start(out=bq_sb, in_=bqD[:, :].rearrange("j p -> p j"))
                nc.sync.dma_start(out=bk_sb, in_=bkD[:, :].rearrange("j p -> p j"))

            # ---- weight / activation slabs (DMA order = first-use order) ----
            w_sbs = {}
            for name in ("q", "k", "v"):
                w_sbs[name] = slabs.tile([128, NCC, cfg.JH], bf16, tag=f"w{name}",
                                         name=f"w{name}_sb")
            x_sb = slabs.tile([128, NCC, cfg.T], bf16)
            xT_v = xT[:, :].rearrange("(cc p) t -> p cc t", p=128)
            # DMA order = first-use order: GK(0,0) runs first (wk, x0),
            # its RoPE tail needs cos/sin, then GQ (wq), then v (wv).
            nc.sync.dma_start(out=w_sbs["k"],
                              in_=wkT[:, :].rearrange("(cc p) j -> p cc j", p=128))
            nc.sync.dma_start(out=x_sb[:, :, 0:TC], in_=xT_v[:, :, 0:TC])
            nc.sync.dma_start(out=cos_sb, in_=cosT[:, :])
            nc.sync.dma_start(out=sin_sb, in_=sinT[:, :])
            nc.sync.dma_start(out=w_sbs["q"],
                              in_=wqT[:, :].rearrange("(cc p) j -> p cc j", p=128))
            nc.sync.dma_start(out=w_sbs["v"],
                              in_=wvT[:, :].rearrange("(cc p) j -> p cc j", p=128))
            for tq in range(1, NTC):
                tql = slice(tq * TC, (tq + 1) * TC)
                nc.sync.dma_start(out=x_sb[:, :, tql], in_=xT_v[:, :, tql])
            wo_sb = slabs.tile([128, NJ, cfg.CO], bf16)
            nc.sync.dma_start(out=wo_sb, in_=woT[:, :].rearrange("(jc p) o -> p jc o", p=128))

            qr_sb = slabs.tile([128, NJ, cfg.T], bf16, tag="qr")
            kr_sb = slabs.tile([128, NJ, cfg.T], bf16, tag="kr")
            v_sb = slabs.tile([128, NT, H, D + 1], bf16, tag="vaug")
            ao_sb = slabs.tile([128, NJ, cfg.T], bf16, tag="ao")
            # ones column for the rowsum trick
            nc.gpsimd.memset(v_sb[:, :, :, D:D + 1], 1.0)

            # ============== granules + deferred-tail scheduler ==============
            # Each granule emits its PE matmuls, then returns a "tail"
            # closure (ops that depend on another engine's evac of those
            # matmuls). Pumped granules run the tail after the NEXT unit's
            # matmuls, so the PE never sits waiting on its own granule's
            # cross-engine round-trip.
            emitted = [0]          # PE rows emitted so far
            pending = []           # granules with un-run tails


            class G:
                __slots__ = ("fn", "rows", "done", "tail")

                def __init__(self, fn, rows):
                    self.fn, self.rows = fn, rows
                    self.done, self.tail = False, None

                def emit(self, defer=False):
                    if self.done:
                        self.flush()
                        return
                    self.done = True
                    tail = self.fn()
                    if tail is not None:
                        if defer:
                            self.tail = tail
                            pending.append(self)
                        else:
                            tail()

                def flush(self):
                    if self.tail is not None:
                        t, self.tail = self.tail, None
                        pending.remove(self)
                        t()

            def flush_pending(except_g=None):
                for g in pending[:]:
                    if g is not except_g:
                        g.flush()

            def proj_granule(name, dst, jt, tcc):
                """k/q projection + RoPE for row-tile jt, t-chunk tcc."""
                tsl = slice(tcc * TC, (tcc + 1) * TC)
                w = w_sbs[name]
                ps = pscr.tile([128, TC], f32, tag="scr", name=f"ps_{name}{jt}_{tcc}")
                for cc in range(NCC):
                    nc.tensor.matmul(
                        ps, lhsT=w[:, cc, jt * 128:(jt + 1) * 128],
                        rhs=x_sb[:, cc, tsl],
                        start=(cc == 0), stop=(cc == NCC - 1))
                if with_qk_bias:
                    b = bq_sb if name == "q" else bk_sb
                    nc.vector.tensor_scalar_add(ps, ps, b[:, jt:jt + 1])
                qb = evac.tile([128, TC], bf16, tag="qb")
                nc.vector.tensor_copy(qb, ps)
                emitted[0] += NCC * TC

                def tail():
                    # RoPE partner fetch: channels are PERM64-ordered, so the
                    # pair of slot i within each 32-partition quadrant is
                    # (i+16)%32 — one DVE stream_shuffle, no PE involved.
                    qs = ropetmp.tile([128, TC], bf16, tag="qs")
                    nc.vector.stream_shuffle(qs, qb, SHUF_MASK)
                    t1 = ropetmp.tile([128, TC], bf16, tag="t1")
                    nc.vector.tensor_mul(t1, qb, cos_sb[:, tsl])
                    t2 = ropetmp.tile([128, TC], bf16, tag="t2")
                    nc.vector.tensor_mul(t2, qs, sin_sb[:, tsl])
                    nc.vector.tensor_add(dst[:, jt, tsl], t1, t2)

                return tail

            def v_granule(tt):
                """v projection (natural orientation) for t-tile tt."""
                ps = pscr.tile([128, cfg.JH], f32, tag="scr", name=f"pv_{tt}")
                for cc in range(NCC):
                    nc.tensor.matmul(
                        ps, lhsT=x_sb[:, cc, tt * 128:(tt + 1) * 128],
                        rhs=w_sbs["v"][:, cc, :],
                        start=(cc == 0), stop=(cc == NCC - 1))
                nc.vector.tensor_copy(
                    v_sb[:, tt, :, 0:D],
                    ps[:, :].rearrange("p (h d) -> p h d", h=H))
                emitted[0] += NCC * cfg.JH

            def outproj_granule(tt):
                """out-projection for t-tile tt (all CO columns)."""
                tq = tt * 128 * NTC // cfg.T
                for p_ in range(NJ):
                    GNB[(tq, p_)].emit()
                ps = psc.tile([128, cfg.CO], f32, tag="psc", name=f"yps_{tt}")
                for u in range(cfg.CO // TC):
                    for jc in range(NJ):
                        nc.tensor.matmul(
                            ps[:, u * TC:(u + 1) * TC],
                            lhsT=ao_sb[:, jc, tt * 128:(tt + 1) * 128],
                            rhs=wo_sb[:, jc, u * TC:(u + 1) * TC],
                            start=(jc == 0), stop=(jc == NJ - 1))
                yb = ypool.tile([128, cfg.CO], f32, tag="yb", name=f"yb_{tt}")
                nc.vector.tensor_copy(yb, ps)
                nc.sync.dma_start(out=y[tt * 128:(tt + 1) * 128, :], in_=yb)
                emitted[0] += (cfg.CO // TC) * NJ * TC

            GK = {(jt, c): G(lambda n=jt, c_=c: proj_granule("k", kr_sb, n, c_),
                             NCC * TC)
                  for jt in range(NJ) for c in range(NTC)}
            GQ = {(jt, c): G(lambda n=jt, c_=c: proj_granule("q", qr_sb, n, c_),
                             NCC * TC)
                  for jt in range(NJ) for c in range(NTC)}
            GV = {tt: G(lambda t_=tt: v_granule(t_), NCC * cfg.JH)
                  for tt in range(NT)}
            GOP = {tt: G(lambda t_=tt: outproj_granule(t_),
                         (cfg.CO // TC) * NJ * TC) for tt in range(NT)}

            # softmax normalize, split in two phases per slot (Act stays
            # exp-only; a [1,N] DVE reciprocal is ~6.5 ns/element, so the
            # rowsum takes a DRAM hop into a [128, TC/128] layout where the
            # reciprocal runs on all 128 lanes in ~50 ns):
            #   A: rowsum -> DRAM -> [128,4] -> DVE reciprocal -> DRAM
            #   B: broadcast 1/r over 64 partitions (DMA), ao = av * rb
            norm_state = {}
            NPF = TC // 128

            def norm_a(avs, pair, tcq):
                st = []
                for half in range(2):
                    rd1 = rdram.tile([1, TC], bf16, tag="rd1", name="rd1")
                    nc.sync.dma_start(out=rd1, in_=avs[half][D:D + 1, :])
                    rtp = rpool.tile([128, NPF], bf16, tag="rtp", name="rtp")
                    nc.sync.dma_start(
                        out=rtp,
                        in_=rd1[0:1, :].rearrange("o (p f) -> (o p) f", p=128))
                    rfp = rpool.tile([128, NPF], f32, tag="rfp", name="rfp")
                    nc.vector.reciprocal(rfp, rtp)
                    rd2 = rdram.tile([1, TC], f32, tag="rd2", name="rd2")
                    nc.sync.dma_start(
                        out=rd2[0:1, :].rearrange("o (p f) -> (o p) f", p=128),
                        in_=rfp)
                    st.append(rd2)
                norm_state[(tcq, pair)] = st

            def norm_b(avs, pair, tcq):
                # broadcast 1/r across 64 partitions (stride-0 DRAM read;
                # off-engine; fully slack -- ao is consumed a slot later)
                tsl = slice(tcq * TC, (tcq + 1) * TC)
                st = norm_state.pop((tcq, pair))
                for half in range(2):
                    p0 = 64 * half
                    rd_ap = st[half][0:1, :]
                    r_bc = bass.AP(
                        tensor=rd_ap.tensor, offset=rd_ap.offset,
                        ap=[[0, D]] + [list(dd) for dd in rd_ap.ap[1:]])
                    rb = rpool.tile([D, TC], f32, tag="rb", name="rb")
                    nc.sync.dma_start(out=rb, in_=r_bc)
                    nc.vector.tensor_mul(
                        ao_sb[p0:p0 + 64, pair, tsl], avs[half][0:D, :], rb)

            GNB = {}

            filler = []            # list of G, pumped in order
            fill_pos = [0]

            def pump(target):
                while emitted[0] < target and fill_pos[0] < len(filler):
                    g = filler[fill_pos[0]]
                    if g.done:
                        fill_pos[0] += 1
                        continue
                    g.emit(defer=True)
                    flush_pending(except_g=g)
                    fill_pos[0] += 1

            # Static filler order: K row-tiles (chunk-major), V tiles, then
            # Q chunks for later tcq; OP granules appended when available.
            for c in range(NTC):
                for jt in range(NJ):
                    filler.append(GK[(jt, c)])
                    if c == 0 and jt > 0:
                        filler.append(GQ[(jt, 0)])
                for k in range(NT // NTC):
                    filler.append(GV[c * (NT // NTC) + k])
            for c in range(1, NTC):
                for jt in range(NJ):
                    filler.append(GQ[(jt, c)])

            total_rows = (2 * NJ * NTC * NCC * TC + NT * NCC * cfg.JH
                          + NSC * NTC * NJ * 2 * TC * 2
                          + NT * (cfg.CO // TC) * NJ * TC)
            per_iter = total_rows // (NTC * NJ * NSC) + 1

            # ---------------- main interleaved slot loop ----------------
            slots = [(tcq, p) for tcq in range(NTC) for p in range(NJ)]

            # prefix: first k/q granules so scores can start immediately
            GK[(0, 0)].emit()
            GQ[(0, 0)].emit()

            prev_slot = None        # (avs, pair, tcq) of previous slot
            for si, (tcq, pair) in enumerate(slots):
                jt = pair
                tsl = slice(tcq * TC, (tcq + 1) * TC)
                if prev_slot is not None:
                    norm_a(*prev_slot)
                    pa, pp_, pt_ = prev_slot
                    GNB[(pt_, pp_)] = G(
                        lambda a=pa, b=pp_, c=pt_: norm_b(a, b, c), 0)
                if si > 0 and si % NJ == 0:
                    prev = tcq - 1
                    for k in range(NT // NTC):
                        filler.append(GOP[prev * (NT // NTC) + k])

                avs = [pav.tile([D + 1, TC], f32, tag="av",
                                name=f"av_{tcq}_{pair}_{i}") for i in range(2)]
                p_tiles = {}
                for sc in range(NSC):
                    GK[(jt, sc * 128 // TC)].emit()
                    GQ[(jt, tcq)].emit()
                    pairP = psc.tile([128, 2 * TC], f32, tag="psc",
                                     name=f"sc_{tcq}_{pair}_{sc}")
                    for half in range(2):
                        p0 = 64 * half
                        nc.tensor.matmul(
                            pairP[:, half * TC:(half + 1) * TC],
                            lhsT=kr_sb[p0:p0 + 64, jt, sc * 128:(sc + 1) * 128],
                            rhs=qr_sb[p0:p0 + 64, jt, tsl],
                            tile_position=(p0, 0))
                    emitted[0] += 2 * TC
                    p_sb = ppool.tile([128, 2 * TC], bf16, tag="p",
                                      name=f"p_{tcq}_{pair}_{sc}")
                    nc.scalar.activation(
                        p_sb, pairP, mybir.ActivationFunctionType.Exp,
                        scale=float(1.0 / np.sqrt(D)))
                    p_tiles[sc] = p_sb
                    flush_pending()
                    if sc == 6 and prev_slot is not None:
                        # free the previous slot's AV psum before our AV starts
                        GNB[(prev_slot[2], prev_slot[1])].emit()
                    if sc >= LAG:
                        sa = sc - LAG
                        GV[sa].emit()
                        pb = p_tiles.pop(sa)
                        for half in range(2):
                            nc.tensor.matmul(
                                avs[half],
                                lhsT=v_sb[:, sa, 2 * pair + half, :],
                                rhs=pb[:, half * TC:(half + 1) * TC],
                                start=(sa == 0), stop=(sa == NSC - 1))
                        emitted[0] += 2 * TC
                        if sa + 1 < NSC:
                            GV[sa + 1].emit()
                    # prefetch upcoming k chunks / next slot deps (deferred)
                    if tcq == 0:
                        nxt = min(sc // 4 + 1, NTC - 1)
                        GK[(jt, nxt)].emit(defer=True)
                    if sc == 13 and si + 1 < len(slots):
                        ntcq, npair = slots[si + 1]
                        if ntcq == 0:
                            GK[(npair, 0)].emit(defer=True)
                        GQ[(npair, ntcq)].emit(defer=True)
                    pump((si * NSC + sc + 1) * per_iter)
                for sa in range(NSC - LAG, NSC):
                    GV[sa].emit()
                    pb = p_tiles.pop(sa)
                    for half in range(2):
                        nc.tensor.matmul(
                            avs[half],
                            lhsT=v_sb[:, sa, 2 * pair + half, :],
                            rhs=pb[:, half * TC:(half + 1) * TC],
                            start=(sa == 0), stop=(sa == NSC - 1))
                    emitted[0] += 2 * TC
                avs_sb = []
                for half in range(2):
                    asb = avsp.tile([D + 1, TC], bf16, tag="avs",
                                    name=f"avsb_{tcq}_{pair}_{half}")
                    nc.vector.tensor_copy(asb, avs[half])
                    avs_sb.append(asb)
                prev_slot = (avs_sb, pair, tcq)

            # tail: last slot's normalize + remaining out-proj
            flush_pending()
            norm_a(*prev_slot)
            GNB[(prev_slot[2], prev_slot[1])] = G(
                lambda a=prev_slot[0], b=prev_slot[1], c=prev_slot[2]:
                norm_b(a, b, c), 0)
            GNB[(prev_slot[2], prev_slot[1])].emit()
            for g in filler:
                g.emit()
            flush_pending()
            for tt in range(NT):
                GOP[tt].emit()

    return nc


_NC_CACHE = {}


def _get_nc(cfg, with_qk_bias):
    key = (cfg.T, cfg.CIN, cfg.JH, cfg.CO, cfg.D, with_qk_bias)
    if key not in _NC_CACHE:
        _NC_CACHE[key] = build_nc(cfg, with_qk_bias)
    return _NC_CACHE[key]


def make_in_maps(cfg, x, Wq, bq, Wk, bk, Wv, bv, Wo, bo, n_groups=2):
    """Build the per-core input dicts. Core c = b * n_groups + g."""
    bf = ml_dtypes.bfloat16
    B = x.shape[0]
    cos, sin = rope_tables(cfg, dtype=bf)
    # PERM64 channel order within each head for q/k (see rope_tables)
    jperm = (np.arange(cfg.JH) // cfg.D) * cfg.D + PERM64[np.arange(cfg.JH) % cfg.D]
    with_qk_bias = bool(np.any(bq) or np.any(bk))
    in_maps = []
    for b in range(B):
        for g in range(n_groups):
            rows = slice(g * cfg.JH, (g + 1) * cfg.JH)
            m = {
                "xT": np.ascontiguousarray(x[b].T).astype(bf),
                "wqT": np.ascontiguousarray(Wq[rows, :][jperm, :].T).astype(bf),
                "wkT": np.ascontiguousarray(Wk[rows, :][jperm, :].T).astype(bf),
                "wvT": np.ascontiguousarray(Wv[rows, :].T).astype(bf),
                "woT": np.ascontiguousarray(Wo[:, rows].T).astype(bf),
                "cosT": cos, "sinT": sin,
            }
            if with_qk_bias:
                m["bq"] = np.ascontiguousarray(
                    bq[rows][jperm].reshape(cfg.NJ, 128).astype(np.float32))
                m["bk"] = np.ascontiguousarray(
                    bk[rows][jperm].reshape(cfg.NJ, 128).astype(np.float32))
            in_maps.append(m)
    return in_maps, with_qk_bias


def run(x, Wq, bq, Wk, bk, Wv, bv, Wo, bo, trace=False):
    from concourse.bass_utils import run_bass_kernel_spmd

    B, T, C = x.shape
    n_groups = 2
    cfg = Cfg(T=T, CIN=C, JH=C // n_groups, CO=C, D=64)
    in_maps, with_qk_bias = make_in_maps(
        cfg, x, Wq, bq, Wk, bk, Wv, bv, Wo, bo, n_groups)
    nc = _get_nc(cfg, with_qk_bias)
    res = run_bass_kernel_spmd(
        nc, in_maps, core_ids=list(range(len(in_maps))), trace=trace)
    out = np.zeros((B, T, C), dtype=np.float32)
    for c, r in enumerate(res.results):
        out[c // n_groups] += r["y"]
    # linear bias corrections (exact): v-bias passes through softmax row-sum=1;
    # out-proj bias is additive.
    out += (bv.astype(np.float32) @ Wo.T.astype(np.float32) + bo.astype(np.float32))
    return out, res


def kernel(x, Wq, bq, Wk, bk, Wv, bv, Wo, bo):
    out, _ = run(
        np.asarray(x, dtype=np.float32),
        np.asarray(Wq, dtype=np.float32), np.asarray(bq, dtype=np.float32),
        np.asarray(Wk, dtype=np.float32), np.asarray(bk, dtype=np.float32),
        np.asarray(Wv, dtype=np.float32), np.asarray(bv, dtype=np.float32),
        np.asarray(Wo, dtype=np.float32), np.asarray(bo, dtype=np.float32))
    return out



# revision 23
# speedup vs baseline: 1.2690x; 1.0093x over previous
"""Multi-head self-attention with RoPE on 8 Trainium2 NeuronCores.

Sharding: core c = batch*2 + head_group. Each core computes 8 of the 16
heads for one batch element end-to-end (QKV projection, RoPE, attention,
out-projection partial). Host sums the two head-group partials per batch
and applies the (linear) bias corrections.

All matmuls run in bf16 with fp32 PSUM accumulation. Softmax skips the
max-subtraction (scores for this problem are bounded by a few units, so
exp() is safe in fp32) and folds the row-sum into the P@V matmul via a
ones-column on V; normalization happens on the tiny [64, t] output.

RoPE's cross-channel pairing uses a PERM64-permuted head-channel order
(pairs 16 partitions apart inside a 32-partition quadrant) so the
partner fetch is a single DVE stream_shuffle, with the permutation
folded into the host-side Wq/Wk row order and cos/sin tables.

Scheduling: a single interleaved emission stream keeps the PE busy
continuously. Projection/V/out-projection granules are pumped between
scores tiles as filler (credit-based pacing), the P@V accumulation lags
the exp by several tiles so the PE never waits on the Act engine, and
the softmax normalization is fully decoupled: the AV accumulator is
evacuated to SBUF (freeing its PSUM bank immediately), the reciprocal
runs on all 128 DVE lanes after a DRAM-hop relayout of the rowsum
(keeping the Act engine exp-only), and the row broadcast is an
off-engine DRAM round-trip.
"""

import numpy as np
import ml_dtypes

# ---------------------------------------------------------------------------
# Workaround: this walrus build rejects >1 sem-wait on a CTRL-only (Drain)
# instruction. TileContext's tail drain carries one wait per outstanding
# logical proc; split them across a chain of single-wait drains.
# ---------------------------------------------------------------------------
_PATCHED = False


def _split_waits_json(raw: bytes) -> bytes:
    """Split instructions carrying >1 sem-wait into single-wait NoOp
    carriers followed by the original instruction (this walrus build
    allows at most one sync-wait per instruction)."""
    import json

    m = json.loads(raw)

    def fix_block(bb):
        insts = bb.get("instructions")
        if not isinstance(insts, list):
            return
        out = []
        for inst in insts:
            si = inst.get("sync_info") if isinstance(inst, dict) else None
            waits = si.get("on_wait") if si else None
            if waits and len(waits) > 1:
                for k, w in enumerate(waits[:-1]):
                    out.append({
                        "debug": inst.get("debug"),
                        "engine": inst["engine"],
                        "ins": [], "outs": [],
                        "name": f'{inst["name"]}_wc{k}',
                        "opcode": "NoOp",
                        "sync_info": {"on_update": [], "on_wait": [w]},
                        "text_hint": "waitsplit",
                    })
                si["on_wait"] = [waits[-1]]
            out.append(inst)
        bb["instructions"] = out

    def walk(obj):
        if isinstance(obj, dict):
            if "instructions" in obj:
                fix_block(obj)
            for v in obj.values():
                walk(v)
        elif isinstance(obj, list):
            for v in obj:
                walk(v)

    walk(m)
    return json.dumps(m).encode()


def _apply_tile_patch():
    global _PATCHED
    if _PATCHED:
        return
    import concourse.bass as bass

    orig = bass.Bass.to_json_bytes

    def to_json_bytes_split(self, *a, **kw):
        return _split_waits_json(orig(self, *a, **kw))

    bass.Bass.to_json_bytes = to_json_bytes_split
    _PATCHED = True


# Note: gpsimd InstISA ops (partition_broadcast) and custom DVE ops
# (reciprocal_approx_fast) fail walrus codegen on this build ("ISA wrong
# length" in visitInstISA) -- only standard BIR ops are used below.


# ---------------------------------------------------------------------------
# Problem dims (hardcoded for the full problem; parameterized for testing)
# ---------------------------------------------------------------------------
class Cfg:
    def __init__(self, T=2048, CIN=1024, JH=512, CO=1024, D=64):
        self.T, self.CIN, self.JH, self.CO, self.D = T, CIN, JH, CO, D
        self.H = JH // D            # heads per core
        self.NCC = CIN // 128       # contraction chunks
        self.NJ = JH // 128         # q/k row tiles
        self.NT = T // 128          # t partition tiles (= s chunks)
        self.TC = 512               # matmul moving-dim chunk
        self.NTC = T // self.TC
        assert JH % 128 == 0 and CIN % 128 == 0 and T % self.TC == 0
        assert D == 64, "RoPE layout assumes D=64 (pairs at +-32 partitions)"


# Head-channel permutation: within each head's 64 channels, order them
# [0..15, 32..47, 16..31, 48..63].  Each RoPE pair (d, d+32) then sits 16
# partitions apart INSIDE one 32-partition quadrant, so the partner fetch
# is a single DVE stream_shuffle (mask i -> (i+16)%32) instead of a PE
# permutation matmul.  Scores contract q.k over d, so any per-head channel
# permutation applied consistently to Wq and Wk rows is exact.
PERM64 = np.concatenate([
    np.arange(0, 16), np.arange(32, 48),
    np.arange(16, 32), np.arange(48, 64)])


def rope_tables(cfg, dtype=np.float32):
    """cos/sin tables for the PERM64-permuted [j-within-tile, t] layout.

    Partition p of a q/k row-tile holds head-channel d = PERM64[p % 64];
    theta index is d % 32.  sin is sign-baked: negative for x1-type
    channels (d < 32, i.e. p%32 < 16), positive for x2-type.
    """
    half = cfg.D // 2
    theta = (10000.0 ** (-np.arange(half, dtype=np.float32) / half)).astype(np.float32)
    t = np.arange(cfg.T, dtype=np.float32)
    freqs = t[None, :] * theta[:, None]          # (32, T) fp32, matches reference
    cos32, sin32 = np.cos(freqs), np.sin(freqs)
    d128 = PERM64[np.arange(128) % 64]
    cos = cos32[d128 % 32, :]                    # (128, T)
    sgn = np.where(d128 < 32, -1.0, 1.0).astype(np.float32)
    sin = sin32[d128 % 32, :] * sgn[:, None]
    return cos.astype(dtype), sin.astype(dtype)


# ---------------------------------------------------------------------------
# Bass program
# ---------------------------------------------------------------------------
def build_nc(cfg, with_qk_bias=False):
    _apply_tile_patch()
    import concourse.bass as bass
    import concourse.tile as tile
    from concourse import mybir
    import contextlib

    f32 = mybir.dt.float32
    bf16 = mybir.dt.bfloat16
    nc = bass.Bass()

    xT = nc.dram_tensor("xT", (cfg.CIN, cfg.T), bf16, kind="ExternalInput")
    wqT = nc.dram_tensor("wqT", (cfg.CIN, cfg.JH), bf16, kind="ExternalInput")
    wkT = nc.dram_tensor("wkT", (cfg.CIN, cfg.JH), bf16, kind="ExternalInput")
    wvT = nc.dram_tensor("wvT", (cfg.CIN, cfg.JH), bf16, kind="ExternalInput")
    woT = nc.dram_tensor("woT", (cfg.JH, cfg.CO), bf16, kind="ExternalInput")
    cosT = nc.dram_tensor("cosT", (128, cfg.T), bf16, kind="ExternalInput")
    sinT = nc.dram_tensor("sinT", (128, cfg.T), bf16, kind="ExternalInput")
    if with_qk_bias:
        bqD = nc.dram_tensor("bq", (cfg.NJ, 128), f32, kind="ExternalInput")
        bkD = nc.dram_tensor("bk", (cfg.NJ, 128), f32, kind="ExternalInput")
    # bf16 partials: host sums the two head-group halves in fp32; the extra
    # ~0.2% rounding on partials is far inside the error budget and halves
    # the output DMA traffic.
    y = nc.dram_tensor("y", (cfg.T, cfg.CO), bf16, kind="ExternalOutput")

    NCC, NJ, NT, TC, NTC, H, D = cfg.NCC, cfg.NJ, cfg.NT, cfg.TC, cfg.NTC, cfg.H, cfg.D
    NSC = NT                      # number of 128-key chunks
    LAG = 7                       # AV lags exp by this many score tiles
    SHUF_MASK = [(i + 16) % 32 for i in range(32)]

    with tile.TileContext(nc) as tc:
        with contextlib.ExitStack() as ctx:
            consts = ctx.enter_context(tc.tile_pool(name="consts", bufs=1))
            slabs = ctx.enter_context(tc.tile_pool(name="slabs", bufs=1))
            evac = ctx.enter_context(tc.tile_pool(name="evac", bufs=3))
            ropetmp = ctx.enter_context(tc.tile_pool(name="ropetmp", bufs=2))
            ppool = ctx.enter_context(tc.tile_pool(name="ppool", bufs=9))
            ypool = ctx.enter_context(tc.tile_pool(name="ypool", bufs=2))
            rpool = ctx.enter_context(tc.tile_pool(name="rpool", bufs=2))
            avsp = ctx.enter_context(tc.tile_pool(name="avsp", bufs=4))
            # rd2 tiles live from norm_a until the next slot's norm_b read,
            # so 2 halves x 2 slots must coexist
            rdram = ctx.enter_context(
                tc.tile_pool(name="rdram", bufs=4, space="DRAM"))
            # PSUM: 4 banks scores/outproj ring + 2 banks AV + 2 scratch
            psc = ctx.enter_context(tc.tile_pool(name="psc", bufs=2, space="PSUM"))
            pav = ctx.enter_context(tc.tile_pool(name="pav", bufs=2, space="PSUM"))
            pscr = ctx.enter_context(tc.tile_pool(name="pscr", bufs=2, space="PSUM"))

            # ---- constants ----
            cos_sb = consts.tile([128, cfg.T], bf16)
            sin_sb = consts.tile([128, cfg.T], bf16)
            if with_qk_bias:
                bq_sb = consts.tile([128, NJ], f32)
                bk_sb = consts.tile([128, NJ], f32)
                nc.sync.dma_start(out=bq_sb, in_=bqD[:, :].rearrange("j p -> p j"))
                nc.sync.dma_start(out=bk_sb, in_=bkD[:, :].rearrange("j p -> p j"))

            # ---- weight / activation slabs (DMA order = first-use order) ----
            w_sbs = {}
            for name in ("q", "k", "v"):
                w_sbs[name] = slabs.tile([128, NCC, cfg.JH], bf16, tag=f"w{name}",
                                         name=f"w{name}_sb")
            x_sb = slabs.tile([128, NCC, cfg.T], bf16)
            xT_v = xT[:, :].rearrange("(cc p) t -> p cc t", p=128)
            # DMA order = first-use order: GK(0,0) runs first (wk, x0),
            # its RoPE tail needs cos/sin, then GQ (wq), then v (wv).
            nc.sync.dma_start(out=w_sbs["k"],
                              in_=wkT[:, :].rearrange("(cc p) j -> p cc j", p=128))
            nc.sync.dma_start(out=x_sb[:, :, 0:TC], in_=xT_v[:, :, 0:TC])
            nc.sync.dma_start(out=cos_sb, in_=cosT[:, :])
            nc.sync.dma_start(out=sin_sb, in_=sinT[:, :])
            nc.sync.dma_start(out=w_sbs["q"],
                              in_=wqT[:, :].rearrange("(cc p) j -> p cc j", p=128))
            nc.sync.dma_start(out=w_sbs["v"],
                              in_=wvT[:, :].rearrange("(cc p) j -> p cc j", p=128))
            for tq in range(1, NTC):
                tql = slice(tq * TC, (tq + 1) * TC)
                nc.sync.dma_start(out=x_sb[:, :, tql], in_=xT_v[:, :, tql])
            wo_sb = slabs.tile([128, NJ, cfg.CO], bf16)
            nc.sync.dma_start(out=wo_sb, in_=woT[:, :].rearrange("(jc p) o -> p jc o", p=128))

            qr_sb = slabs.tile([128, NJ, cfg.T], bf16, tag="qr")
            kr_sb = slabs.tile([128, NJ, cfg.T], bf16, tag="kr")
            v_sb = slabs.tile([128, NT, H, D + 1], bf16, tag="vaug")
            ao_sb = slabs.tile([128, NJ, cfg.T], bf16, tag="ao")
            # ones column for the rowsum trick
            nc.gpsimd.memset(v_sb[:, :, :, D:D + 1], 1.0)

            # ============== granules + deferred-tail scheduler ==============
            # Each granule emits its PE matmuls, then returns a "tail"
            # closure (ops that depend on another engine's evac of those
            # matmuls). Pumped granules run the tail after the NEXT unit's
            # matmuls, so the PE never sits waiting on its own granule's
            # cross-engine round-trip.
            emitted = [0]          # PE rows emitted so far
            pending = []           # granules with un-run tails


            class G:
                __slots__ = ("fn", "rows", "done", "tail")

                def __init__(self, fn, rows):
                    self.fn, self.rows = fn, rows
                    self.done, self.tail = False, None

                def emit(self, defer=False):
                    if self.done:
                        self.flush()
                        return
                    self.done = True
                    tail = self.fn()
                    if tail is not None:
                        if defer:
                            self.tail = tail
                            pending.append(self)
                        else:
                            tail()

                def flush(self):
                    if self.tail is not None:
                        t, self.tail = self.tail, None
                        pending.remove(self)
                        t()

            def flush_pending(except_g=None):
                for g in pending[:]:
                    if g is not except_g:
                        g.flush()

            def proj_granule(name, dst, jt, tcc):
                """k/q projection + RoPE for row-tile jt, t-chunk tcc."""
                tsl = slice(tcc * TC, (tcc + 1) * TC)
                w = w_sbs[name]
                ps = pscr.tile([128, TC], f32, tag="scr", name=f"ps_{name}{jt}_{tcc}")
                for cc in range(NCC):
                    nc.tensor.matmul(
                        ps, lhsT=w[:, cc, jt * 128:(jt + 1) * 128],
                        rhs=x_sb[:, cc, tsl],
                        start=(cc == 0), stop=(cc == NCC - 1))
                if with_qk_bias:
                    b = bq_sb if name == "q" else bk_sb
                    nc.vector.tensor_scalar_add(ps, ps, b[:, jt:jt + 1])
                qb = evac.tile([128, TC], bf16, tag="qb")
                nc.vector.tensor_copy(qb, ps)
                emitted[0] += NCC * TC

                def tail():
                    # RoPE partner fetch: channels are PERM64-ordered, so the
                    # pair of slot i within each 32-partition quadrant is
                    # (i+16)%32 — one DVE stream_shuffle, no PE involved.
                    qs = ropetmp.tile([128, TC], bf16, tag="qs")
                    nc.vector.stream_shuffle(qs, qb, SHUF_MASK)
                    t1 = ropetmp.tile([128, TC], bf16, tag="t1")
                    nc.vector.tensor_mul(t1, qb, cos_sb[:, tsl])
                    t2 = ropetmp.tile([128, TC], bf16, tag="t2")
                    nc.vector.tensor_mul(t2, qs, sin_sb[:, tsl])
                    nc.vector.tensor_add(dst[:, jt, tsl], t1, t2)

                return tail

            def v_granule(tt):
                """v projection (natural orientation) for t-tile tt."""
                ps = pscr.tile([128, cfg.JH], f32, tag="scr", name=f"pv_{tt}")
                for cc in range(NCC):
                    nc.tensor.matmul(
                        ps, lhsT=x_sb[:, cc, tt * 128:(tt + 1) * 128],
                        rhs=w_sbs["v"][:, cc, :],
                        start=(cc == 0), stop=(cc == NCC - 1))
                nc.vector.tensor_copy(
                    v_sb[:, tt, :, 0:D],
                    ps[:, :].rearrange("p (h d) -> p h d", h=H))
                emitted[0] += NCC * cfg.JH

            def outproj_granule(tt):
                """out-projection for t-tile tt (all CO columns)."""
                tq = tt * 128 * NTC // cfg.T
                for p_ in range(NJ):
                    GNB[(tq, p_)].emit()
                ps = psc.tile([128, cfg.CO], f32, tag="psc", name=f"yps_{tt}")
                for u in range(cfg.CO // TC):
                    for jc in range(NJ):
                        nc.tensor.matmul(
                            ps[:, u * TC:(u + 1) * TC],
                            lhsT=ao_sb[:, jc, tt * 128:(tt + 1) * 128],
                            rhs=wo_sb[:, jc, u * TC:(u + 1) * TC],
                            start=(jc == 0), stop=(jc == NJ - 1))
                yb = ypool.tile([128, cfg.CO], bf16, tag="yb", name=f"yb_{tt}")
                nc.vector.tensor_copy(yb, ps)
                nc.sync.dma_start(out=y[tt * 128:(tt + 1) * 128, :], in_=yb)
                emitted[0] += (cfg.CO // TC) * NJ * TC

            GK = {(jt, c): G(lambda n=jt, c_=c: proj_granule("k", kr_sb, n, c_),
                             NCC * TC)
                  for jt in range(NJ) for c in range(NTC)}
            GQ = {(jt, c): G(lambda n=jt, c_=c: proj_granule("q", qr_sb, n, c_),
                             NCC * TC)
                  for jt in range(NJ) for c in range(NTC)}
            GV = {tt: G(lambda t_=tt: v_granule(t_), NCC * cfg.JH)
                  for tt in range(NT)}
            GOP = {tt: G(lambda t_=tt: outproj_granule(t_),
                         (cfg.CO // TC) * NJ * TC) for tt in range(NT)}

            # softmax normalize, split in two phases per slot (Act stays
            # exp-only; a [1,N] DVE reciprocal is ~6.5 ns/element, so the
            # rowsum takes a DRAM hop into a [128, TC/128] layout where the
            # reciprocal runs on all 128 lanes in ~50 ns):
            #   A: rowsum -> DRAM -> [128,4] -> DVE reciprocal -> DRAM
            #   B: broadcast 1/r over 64 partitions (DMA), ao = av * rb
            norm_state = {}
            NPF = TC // 128

            def norm_a(avs, pair, tcq, fast=False):
                st = []
                for half in range(2):
                    if fast:
                        # tail fast-path: Act is idle once the slots end, and
                        # ln+exp avoids the 3-hop DMA latency chain
                        rln = rpool.tile([1, TC], f32, tag="rln", name="rln")
                        nc.scalar.activation(
                            rln, avs[half][D:D + 1, :],
                            mybir.ActivationFunctionType.Ln)
                        rf = rpool.tile([1, TC], f32, tag="rf", name="rf")
                        nc.scalar.activation(
                            rf, rln, mybir.ActivationFunctionType.Exp,
                            scale=-1.0)
                        rd2 = rdram.tile([1, TC], f32, tag="rd2", name="rd2")
                        nc.sync.dma_start(out=rd2, in_=rf)
                        st.append(rd2)
                        continue
                    rd1 = rdram.tile([1, TC], bf16, tag="rd1", name="rd1")
                    nc.sync.dma_start(out=rd1, in_=avs[half][D:D + 1, :])
                    rtp = rpool.tile([128, NPF], bf16, tag="rtp", name="rtp")
                    nc.sync.dma_start(
                        out=rtp,
                        in_=rd1[0:1, :].rearrange("o (p f) -> (o p) f", p=128))
                    rfp = rpool.tile([128, NPF], f32, tag="rfp", name="rfp")
                    nc.vector.reciprocal(rfp, rtp)
                    rd2 = rdram.tile([1, TC], f32, tag="rd2", name="rd2")
                    nc.sync.dma_start(
                        out=rd2[0:1, :].rearrange("o (p f) -> (o p) f", p=128),
                        in_=rfp)
                    st.append(rd2)
                norm_state[(tcq, pair)] = st

            def norm_b(avs, pair, tcq):
                # broadcast 1/r across 64 partitions (stride-0 DRAM read;
                # off-engine; fully slack -- ao is consumed a slot later)
                tsl = slice(tcq * TC, (tcq + 1) * TC)
                st = norm_state.pop((tcq, pair))
                for half in range(2):
                    p0 = 64 * half
                    rd_ap = st[half][0:1, :]
                    r_bc = bass.AP(
                        tensor=rd_ap.tensor, offset=rd_ap.offset,
                        ap=[[0, D]] + [list(dd) for dd in rd_ap.ap[1:]])
                    rb = rpool.tile([D, TC], f32, tag="rb", name="rb")
                    nc.sync.dma_start(out=rb, in_=r_bc)
                    nc.vector.tensor_mul(
                        ao_sb[p0:p0 + 64, pair, tsl], avs[half][0:D, :], rb)

            GNB = {}

            filler = []            # list of G, pumped in order
            fill_pos = [0]

            def pump(target):
                while emitted[0] < target and fill_pos[0] < len(filler):
                    g = filler[fill_pos[0]]
                    if g.done:
                        fill_pos[0] += 1
                        continue
                    g.emit(defer=True)
                    flush_pending(except_g=g)
                    fill_pos[0] += 1

            # Static filler order: K row-tiles (chunk-major), V tiles, then
            # Q chunks for later tcq; OP granules appended when available.
            for c in range(NTC):
                for jt in range(NJ):
                    filler.append(GK[(jt, c)])
                    if c == 0 and jt > 0:
                        filler.append(GQ[(jt, 0)])
                for k in range(NT // NTC):
                    filler.append(GV[c * (NT // NTC) + k])
            for c in range(1, NTC):
                for jt in range(NJ):
                    filler.append(GQ[(jt, c)])

            total_rows = (2 * NJ * NTC * NCC * TC + NT * NCC * cfg.JH
                          + NSC * NTC * NJ * 2 * TC * 2
                          + NT * (cfg.CO // TC) * NJ * TC)
            per_iter = total_rows // (NTC * NJ * NSC) + 1

            # ---------------- main interleaved slot loop ----------------
            slots = [(tcq, p) for tcq in range(NTC) for p in range(NJ)]

            # prefix: k granules first (they need only wk+x0, the first two
            # DMAs) — they keep the PE busy while wq is still in flight;
            # GQ(0,0) lands just as wq arrives.
            GK[(0, 0)].emit()
            GK[(1, 0)].emit()
            GK[(2, 0)].emit()
            GQ[(0, 0)].emit()

            prev_slot = None        # (avs, pair, tcq) of previous slot
            for si, (tcq, pair) in enumerate(slots):
                jt = pair
                tsl = slice(tcq * TC, (tcq + 1) * TC)
                if prev_slot is not None:
                    norm_a(*prev_slot)
                    pa, pp_, pt_ = prev_slot
                    GNB[(pt_, pp_)] = G(
                        lambda a=pa, b=pp_, c=pt_: norm_b(a, b, c), 0)
                if si > 0 and si % NJ == 0:
                    prev = tcq - 1
                    for k in range(NT // NTC):
                        filler.append(GOP[prev * (NT // NTC) + k])

                avs = [pav.tile([D + 1, TC], f32, tag="av",
                                name=f"av_{tcq}_{pair}_{i}") for i in range(2)]
                p_tiles = {}
                for sc in range(NSC):
                    GK[(jt, sc * 128 // TC)].emit()
                    GQ[(jt, tcq)].emit()
                    pairP = psc.tile([128, 2 * TC], f32, tag="psc",
                                     name=f"sc_{tcq}_{pair}_{sc}")
                    for half in range(2):
                        p0 = 64 * half
                        nc.tensor.matmul(
                            pairP[:, half * TC:(half + 1) * TC],
                            lhsT=kr_sb[p0:p0 + 64, jt, sc * 128:(sc + 1) * 128],
                            rhs=qr_sb[p0:p0 + 64, jt, tsl],
                            tile_position=(p0, 0))
                    emitted[0] += 2 * TC
                    p_sb = ppool.tile([128, 2 * TC], bf16, tag="p",
                                      name=f"p_{tcq}_{pair}_{sc}")
                    nc.scalar.activation(
                        p_sb, pairP, mybir.ActivationFunctionType.Exp,
                        scale=float(1.0 / np.sqrt(D)))
                    p_tiles[sc] = p_sb
                    flush_pending()
                    if sc == 6 and prev_slot is not None:
                        # free the previous slot's AV psum before our AV starts
                        GNB[(prev_slot[2], prev_slot[1])].emit()
                    if sc >= LAG:
                        sa = sc - LAG
                        GV[sa].emit()
                        pb = p_tiles.pop(sa)
                        for half in range(2):
                            nc.tensor.matmul(
                                avs[half],
                                lhsT=v_sb[:, sa, 2 * pair + half, :],
                                rhs=pb[:, half * TC:(half + 1) * TC],
                                start=(sa == 0), stop=(sa == NSC - 1))
                        emitted[0] += 2 * TC
                        if sa + 1 < NSC:
                            GV[sa + 1].emit()
                    # prefetch upcoming k chunks / next slot deps (deferred)
                    if tcq == 0:
                        nxt = min(sc // 4 + 1, NTC - 1)
                        GK[(jt, nxt)].emit(defer=True)
                    if sc == 13 and si + 1 < len(slots):
                        ntcq, npair = slots[si + 1]
                        if ntcq == 0:
                            GK[(npair, 0)].emit(defer=True)
                        GQ[(npair, ntcq)].emit(defer=True)
                    pump((si * NSC + sc + 1) * per_iter)
                for sa in range(NSC - LAG, NSC):
                    GV[sa].emit()
                    pb = p_tiles.pop(sa)
                    for half in range(2):
                        nc.tensor.matmul(
                            avs[half],
                            lhsT=v_sb[:, sa, 2 * pair + half, :],
                            rhs=pb[:, half * TC:(half + 1) * TC],
                            start=(sa == 0), stop=(sa == NSC - 1))
                    emitted[0] += 2 * TC
                avs_sb = []
                for half in range(2):
                    asb = avsp.tile([D + 1, TC], bf16, tag="avs",
                                    name=f"avsb_{tcq}_{pair}_{half}")
                    nc.vector.tensor_copy(asb, avs[half])
                    avs_sb.append(asb)
                prev_slot = (avs_sb, pair, tcq)

            # tail: last slot's normalize + remaining out-proj
            flush_pending()
            norm_a(*prev_slot, fast=True)
            GNB[(prev_slot[2], prev_slot[1])] = G(
                lambda a=prev_slot[0], b=prev_slot[1], c=prev_slot[2]:
                norm_b(a, b, c), 0)
            GNB[(prev_slot[2], prev_slot[1])].emit()
            for g in filler:
                g.emit()
            flush_pending()
            for tt in range(NT):
                GOP[tt].emit()

    return nc


_NC_CACHE = {}


def _get_nc(cfg, with_qk_bias):
    key = (cfg.T, cfg.CIN, cfg.JH, cfg.CO, cfg.D, with_qk_bias)
    if key not in _NC_CACHE:
        _NC_CACHE[key] = build_nc(cfg, with_qk_bias)
    return _NC_CACHE[key]


def make_in_maps(cfg, x, Wq, bq, Wk, bk, Wv, bv, Wo, bo, n_groups=2):
    """Build the per-core input dicts. Core c = b * n_groups + g."""
    bf = ml_dtypes.bfloat16
    B = x.shape[0]
    cos, sin = rope_tables(cfg, dtype=bf)
    # PERM64 channel order within each head for q/k (see rope_tables)
    jperm = (np.arange(cfg.JH) // cfg.D) * cfg.D + PERM64[np.arange(cfg.JH) % cfg.D]
    with_qk_bias = bool(np.any(bq) or np.any(bk))
    in_maps = []
    for b in range(B):
        for g in range(n_groups):
            rows = slice(g * cfg.JH, (g + 1) * cfg.JH)
            m = {
                "xT": np.ascontiguousarray(x[b].T).astype(bf),
                "wqT": np.ascontiguousarray(Wq[rows, :][jperm, :].T).astype(bf),
                "wkT": np.ascontiguousarray(Wk[rows, :][jperm, :].T).astype(bf),
                "wvT": np.ascontiguousarray(Wv[rows, :].T).astype(bf),
                "woT": np.ascontiguousarray(Wo[:, rows].T).astype(bf),
                "cosT": cos, "sinT": sin,
            }
            if with_qk_bias:
                m["bq"] = np.ascontiguousarray(
                    bq[rows][jperm].reshape(cfg.NJ, 128).astype(np.float32))
                m["bk"] = np.ascontiguousarray(
                    bk[rows][jperm].reshape(cfg.NJ, 128).astype(np.float32))
            in_maps.append(m)
    return in_maps, with_qk_bias


def run(x, Wq, bq, Wk, bk, Wv, bv, Wo, bo, trace=False):
    from concourse.bass_utils import run_bass_kernel_spmd

    B, T, C = x.shape
    n_groups = 2
    cfg = Cfg(T=T, CIN=C, JH=C // n_groups, CO=C, D=64)
    in_maps, with_qk_bias = make_in_maps(
        cfg, x, Wq, bq, Wk, bk, Wv, bv, Wo, bo, n_groups)
    nc = _get_nc(cfg, with_qk_bias)
    res = run_bass_kernel_spmd(
        nc, in_maps, core_ids=list(range(len(in_maps))), trace=trace)
    out = np.zeros((B, T, C), dtype=np.float32)
    for c, r in enumerate(res.results):
        out[c // n_groups] += np.asarray(r["y"], dtype=np.float32)
    # linear bias corrections (exact): v-bias passes through softmax row-sum=1;
    # out-proj bias is additive.
    out += (bv.astype(np.float32) @ Wo.T.astype(np.float32) + bo.astype(np.float32))
    return out, res


def kernel(x, Wq, bq, Wk, bk, Wv, bv, Wo, bo):
    out, _ = run(
        np.asarray(x, dtype=np.float32),
        np.asarray(Wq, dtype=np.float32), np.asarray(bq, dtype=np.float32),
        np.asarray(Wk, dtype=np.float32), np.asarray(bk, dtype=np.float32),
        np.asarray(Wv, dtype=np.float32), np.asarray(bv, dtype=np.float32),
        np.asarray(Wo, dtype=np.float32), np.asarray(bo, dtype=np.float32))
    return out

